# revision 8
# baseline (speedup 1.0000x reference)
"""Trainium2 Bass kernel for CLSProcess: diagonal linear recurrence
state_t = y_t * state_{t-1} + x_t * z_t over [B=8, T=4096, units=1024].

Sharding: batch across the 8 cores (one batch element per core).

v2 design (vs v1 baseline at ~143-162us):
  - output written to DRAM in bf16 (8.4 MB instead of 16.8 MB per core);
    host upconverts to fp32. Total HBM traffic/core: 16.8 in + 8.4 out
    = 25.2 MB -> ~70us floor at 16 DMA engines x 22.5 B/ns.
  - y rows are DMA-gathered straight from DRAM into a single
    [1, 4096] row (zeroed once at block starts for the scan reset),
    removing the per-block PE transpose + scalar copy of v1.
  - the x gate is folded into the decay matrix M (M' = diag-row scale
    of M by x, one [128,128] activation per block) instead of scaling
    z ([128,1024] per block); the matmul rhs is the raw f32r input
    (f32r moving operand with 512 columns streams at 1 cycle/row).
  - single PSUM->SBUF drain per block into a bf16 tile that serves as
    both the carry operand and the output-DMA source (v1 drained twice,
    fp32 + bf16).
  - input DMAs dispatch from the SP (sync) HWDGE queue, output DMAs
    from the Activation HWDGE queue, so output-drain waits never stall
    input prefetch.
"""

import numpy as np

import concourse.bacc as bacc
import concourse.bass as bass
import concourse.mybir as mybir
import concourse.tile as tile
from concourse.bass_utils import run_bass_kernel_spmd

B = 8
T = 4096
F = 1026
U = 1024
L = 128
G = 4  # blocks per scan batch
f32 = mybir.dt.float32
f32r = mybir.dt.float32r
bf16 = mybir.dt.bfloat16


def build_nc(t_total: int = T) -> bass.Bass:
    nb = t_total // L
    ng = (nb + G - 1) // G
    nc = bacc.Bacc()
    inp = nc.dram_tensor("inp", [t_total, F], f32, kind="ExternalInput")
    out = nc.dram_tensor("out", [t_total, U], bf16, kind="ExternalOutput")
    ident4_d = nc.inline_tensor(
        np.tile(np.eye(L, dtype=np.float32), (1, G)), name="ident4"
    )
    e127c_np = np.zeros((L, 1), dtype=np.float32)
    e127c_np[L - 1, 0] = 1.0
    e127c_d = nc.inline_tensor(e127c_np, name="e127c")

    with tile.TileContext(nc) as tc:
        with (
            tc.tile_pool(name="const", bufs=1) as constp,
            tc.tile_pool(name="yrow", bufs=1) as yrowp,
            tc.tile_pool(name="inpool", bufs=14) as inpool,
            tc.tile_pool(name="mpool", bufs=2) as mpool,
            tc.tile_pool(name="mscpool", bufs=4) as mscpool,
            tc.tile_pool(name="rowpool", bufs=2) as rowpool,
            tc.tile_pool(name="bcpool", bufs=3) as bcpool,
            tc.tile_pool(name="pbpool", bufs=2) as pbpool,
            tc.tile_pool(name="selpool", bufs=3) as selpool,
            tc.tile_pool(name="otbpool", bufs=3) as otbpool,
            tc.tile_pool(name="ps_out", bufs=4, space="PSUM") as ps_out_pool,
        ):
            ident4 = constp.tile([L, G * L], f32, tag="ident4")
            nc.sync.dma_start(ident4[:], ident4_d[:, :])
            e127c = constp.tile([L, 1], f32, tag="e127c")
            nc.sync.dma_start(e127c[:], e127c_d[:, :])

            # all block-start y positions stay 0 => the scan state resets
            yrowall = yrowp.tile([1, t_total], f32, tag="yrowall")
            nc.gpsimd.memset(yrowall[:], 0.0)

            tins = {}
            ybcs = {}

            def dispatch_group(g: int):
                ks = range(g * G, min((g + 1) * G, nb))
                for k in ks:
                    r0 = k * L
                    tin = inpool.tile([L, F], f32r, tag="tin")
                    nc.sync.dma_start(tin[:], inp[r0 : r0 + L, :].bitcast(f32r))
                    tins[k] = tin
                for k in ks:
                    r0 = k * L
                    nc.sync.dma_start(
                        yrowall[0:1, k * L + 1 : (k + 1) * L],
                        inp[r0 + 1 : r0 + L, 1:2].rearrange("a b -> b a"),
                    )
                ybc = bcpool.tile([L, G * L], f32, tag="ybc")
                nc.gpsimd.partition_broadcast(
                    ybc[:], yrowall[0:1, g * G * L : (g + 1) * G * L]
                )
                ybcs[g] = ybc

            dispatch_group(0)
            prev = None
            for g in range(ng):
                if g + 1 < ng:
                    dispatch_group(g + 1)
                # mt4[s, L*j + t] = M_{block}[t, s] (prod_{r=s+1..t} y_r)
                mt4 = mpool.tile([L, G * L], f32r, tag="mt4")
                nc.vector.tensor_tensor_scan(
                    mt4[:],
                    ybcs.pop(g)[:],
                    ident4[:],
                    0.0,
                    mybir.AluOpType.mult,
                    mybir.AluOpType.add,
                )
                for j, k in enumerate(range(g * G, min((g + 1) * G, nb))):
                    r0 = k * L
                    tin = tins.pop(k)
                    mtk = mt4[:, L * j : L * j + L]

                    # msc[s, t] = x_s * M[t, s] (f32r: walrus forbids mixing
                    # 32-bit lhsT with non-32-bit rhs, and rhs is the raw
                    # f32 input; f32r x f32r still streams 1 col/cycle)
                    msc = mscpool.tile([L, L], f32r, tag="msc")
                    nc.scalar.activation(
                        msc[:],
                        mtk,
                        mybir.ActivationFunctionType.Copy,
                        scale=tin[:, 0:1].bitcast(f32),
                    )

                    if k > 0:
                        # p_t = prod_{r=block_start..t} y_r = y_0 * mt[0, t]
                        prow = rowpool.tile([1, L], f32, tag="prow")
                        nc.vector.tensor_scalar_mul(
                            prow[:], mtk[0:1, :], tin[0:1, 1:2].bitcast(f32)
                        )
                        # sel[s, t] = I[s==127] * p_t
                        pb = pbpool.tile([L, L], f32, tag="pb")
                        nc.gpsimd.partition_broadcast(pb[:], prow[0:1, :])
                        sel = selpool.tile([L, L], bf16, tag="sel")
                        nc.vector.tensor_scalar_mul(sel[:], pb[:], e127c[:])

                    po = ps_out_pool.tile([L, U], f32, tag="po")
                    for jj in (0, 512):
                        nc.tensor.matmul(
                            po[:, jj : jj + 512],
                            msc[:],
                            tin[:, 2 + jj : 2 + jj + 512],
                            start=True,
                            stop=(k == 0),
                        )
                    if k > 0:
                        # po[t, :] += p_t * prev[127, :]
                        for jj in (0, 512):
                            nc.tensor.matmul(
                                po[:, jj : jj + 512],
                                sel[:],
                                prev[:, jj : jj + 512],
                                start=False,
                                stop=True,
                            )
                    # single bf16 drain: carry operand + output-DMA source
                    otb = otbpool.tile([L, U], bf16, tag="otb")
                    nc.vector.tensor_copy(otb[:, 0:384], po[:, 0:384])
                    nc.scalar.copy(otb[:, 384:1024], po[:, 384:1024])
                    nc.scalar.dma_start(out[r0 : r0 + L, :], otb[:])
                    prev = otb
    nc.finalize()
    return nc


_NC = None


def _get_nc() -> bass.Bass:
    global _NC
    if _NC is None:
        _NC = build_nc()
    return _NC


def kernel(**inputs: np.ndarray) -> np.ndarray:
    x = np.ascontiguousarray(inputs["inputs"], dtype=np.float32)
    assert x.shape == (B, T, F), x.shape
    nc = _get_nc()
    in_maps = [{"inp": x[c]} for c in range(B)]
    res = run_bass_kernel_spmd(nc, in_maps, core_ids=list(range(B)))
    return np.stack(
        [np.asarray(res.results[c]["out"]).astype(np.float32) for c in range(B)],
        axis=0,
    )
